# revision 33
# baseline (speedup 1.0000x reference)
"""Distributed causal multi-head attention with cumulative relative-position
bias for Trainium2 (8 NeuronCores).

Problem: x:[2,2048,1024], qkv:[1024,3,16,64], out_w:[16,64,1024],
rpe_bias:[16,2048] -> out:[2,2048,1024]

Sharding: data-parallel over batch (2) x tensor-parallel over head groups
(16 heads -> 4 groups of 4). Core c handles batch c//4, heads [4*(c%4), 4*(c%4)+4).
Each core emits a partial output [2048,1024] (bf16); the host sums the 4
head-group partials per batch (the "all-reduce" of the out projection).

Math tricks:
 - 1/sqrt(D) folded into the Q projection weights on host.
 - bias[i,j] = cumsum(rpe)[i-j] (i>=j, else -inf) is Toeplitz:
   exp(S + bias) = exp(S) * G where G[k_row, q_col] = exp(cum)[q-k] (0 above
   the diagonal). A single [128, 512+2048] bf16 tile per head serves every
   128-row band of the score matrix.
 - Scores are computed transposed (S^T[k,q] = K_h Q_h^T) so that P^T feeds
   the PV matmul directly (lhsT = V natural layout) with no transposes.
 - A ones-column appended to V yields softmax row-sums for free in the same
   matmul; normalization is folded into the PSUM->SBUF eviction of mix^T.

Schedule (v5): attention iterates k-block PAIRS into a two-region
[128,1024] score band so each pair costs one exp + one G-multiply
(an overlapping strided AP covers both regions' G slices).  Scores/exp
are causally exact at 128 granularity; the G=0 region zeroes the
uncomputed prefix columns of pband so PV can run 512-aligned (middle
diagonal PV matmuls also skip their zero prefix).  Per-pair the PE waits
~0.3-0.5us on the Act engine's exp, which drops it out of max p-state;
projection and output-projection matmuls are therefore interleaved as
FILLER STEPS inside the attention pair loop (own 2-bank PSUM buffer,
evictions on DVE so a filler never blocks PE behind the Act queue).
Input DMAs split across the GpSimd (x) / SP (w_qk) / Act (rest) queues;
softmax-normalization DMA chains alternate SP/GpSimd queues.
"""

import sys

if "/opt/trn_rl_repo" not in sys.path:
    sys.path.insert(0, "/opt/trn_rl_repo")

import numpy as np
import ml_dtypes

B, S, HID, NH, D = 2, 2048, 1024, 16, 64
NCORES = 8
HPC = 4  # heads per core
KB = 16  # 128-row k blocks
BF16 = ml_dtypes.bfloat16

_CACHE = {}


def build_nc():
    import concourse.mybir as mybir
    from concourse import bacc
    from concourse.tile import TileContext
    from concourse.ap import AP

    f32 = mybir.dt.float32
    bf16 = mybir.dt.bfloat16
    Exp = mybir.ActivationFunctionType.Exp
    Copy = mybir.ActivationFunctionType.Copy

    nc = bacc.Bacc()

    x_t = nc.declare_dram_parameter("x_t", [HID, S], bf16, isOutput=False)
    w_qk = nc.declare_dram_parameter("w_qk", [HID, 512], bf16, isOutput=False)
    w_v = nc.declare_dram_parameter("w_v", [HID, 260], bf16, isOutput=False)
    g_ext = nc.declare_dram_parameter("g_ext", [HPC, 128, 512 + S], bf16, isOutput=False)
    w_out = nc.declare_dram_parameter("w_out", [256, HID], bf16, isOutput=False)
    out = nc.declare_dram_parameter("out", [S, HID], bf16, isOutput=True)

    with TileContext(nc) as tc:
        with (
            tc.tile_pool(name="persist", bufs=1) as persist,
            tc.tile_pool(name="work", bufs=3) as work,
            tc.tile_pool(name="work2", bufs=2) as work2,
            tc.tile_pool(name="dram", bufs=2, space="DRAM") as dpool,
            tc.tile_pool(name="psum", bufs=2, space="PSUM") as psum,
        ):
            # warm the exp activation-table during the input DMA wait
            warm = persist.tile([1, 8], f32, tag="warm", name="warm")
            nc.vector.memset(warm, 0.0)
            nc.scalar.activation(out=warm, in_=warm, func=Exp)

            # input DMAs: wqk + first x halves on the GpSimd queue (boots
            # ~3us before SP), second x halves on SP, rest on the Act queue.
            # The first projection pass and V st0-7 only touch x cols [0,1024),
            # so attention can start before the second halves land.
            wqk_sb, xt_sb = [], []
            for i in range(8):
                tw = persist.tile([128, 512], bf16, tag=f"wqk{i}", name=f"wqk{i}")
                nc.gpsimd.dma_start(out=tw, in_=w_qk[i * 128 : (i + 1) * 128, :])
                wqk_sb.append(tw)
                tx = persist.tile([128, S], bf16, tag=f"xt{i}", name=f"xt{i}")
                nc.gpsimd.dma_start(out=tx[:, 0:1024], in_=x_t[i * 128 : (i + 1) * 128, 0:1024])
                xt_sb.append(tx)
            for i in range(8):
                nc.gpsimd.dma_start(out=xt_sb[i][:, 1024:S], in_=x_t[i * 128 : (i + 1) * 128, 1024:S])
            g_sb = [
                persist.tile([128, 512 + S], bf16, tag=f"g{h}", name=f"g{h}")
                for h in range(HPC)
            ]
            wv_sb = [
                persist.tile([128, 260], bf16, tag=f"wv{i}", name=f"wv{i}")
                for i in range(8)
            ]
            wout_sb = [
                persist.tile([128, HID], bf16, tag=f"wout{i}", name=f"wout{i}")
                for i in range(2)
            ]
            for i in range(8):
                nc.scalar.dma_start(out=wv_sb[i], in_=w_v[i * 128 : (i + 1) * 128, :])
            nc.scalar.dma_start(out=g_sb[0], in_=g_ext[0])
            nc.scalar.dma_start(out=g_sb[1], in_=g_ext[1])
            for i in range(2):
                nc.scalar.dma_start(out=wout_sb[i], in_=w_out[i * 128 : (i + 1) * 128, :])
            for h in range(2, HPC):
                nc.scalar.dma_start(out=g_sb[h], in_=g_ext[h])

            qk_sb = [persist.tile([128, S], bf16, tag=f"qk{mt}", name=f"qk{mt}") for mt in range(4)]
            v_sb = [persist.tile([128, 260], bf16, tag=f"v{st}", name=f"v{st}") for st in range(KB)]
            mixT_sb = [persist.tile([128, S], bf16, tag=f"mixT{i}", name=f"mixT{i}") for i in range(2)]

            # pexp buffers rotate; stale prefix columns are multiplied by G=0,
            # so they only need to hold FINITE values -> zero them once.
            for i in range(3):
                t = work.tile([128, 1024], bf16, tag="pexp", name="pexpinit")
                nc.vector.memset(t, 0.0)

            # identity + ones for the PE-based final normalization chain
            from concourse.masks import make_identity

            ident128 = persist.tile([128, 128], f32, tag="ident", name="ident")
            make_identity(nc, ident128)
            ones64 = persist.tile([1, 64], f32, tag="ones64", name="ones64")
            nc.vector.memset(ones64, 1.0)

            state = {"chain": 0}

            def qk_pass(mt, half, tag="fill", evict_act=False):
                """Generator: one yield per matmul; eviction per 512-half.
                Single-bank fill tiles with bufs=2 let consecutive filler
                tasks overlap instead of serializing on each eviction."""
                for c in range(2):
                    ps = psum.tile([128, 512], f32, tag="fill", name="qkps", bufs=2)
                    for xc in range(8):
                        nc.tensor.matmul(
                            ps,
                            lhsT=wqk_sb[xc][:, mt * 128 : (mt + 1) * 128],
                            rhs=xt_sb[xc][:, half * 1024 + c * 512 : half * 1024 + (c + 1) * 512],
                            start=(xc == 0),
                            stop=(xc == 7),
                        )
                        yield
                    dst = qk_sb[mt][:, half * 1024 + c * 512 : half * 1024 + (c + 1) * 512]
                    if evict_act:
                        nc.scalar.activation(out=dst, in_=ps, func=Copy)
                    else:
                        nc.vector.tensor_copy(out=dst, in_=ps)

            def v_pass(st, tag="fill"):
                ps = psum.tile([128, 512], f32, tag="fill", name="vps", bufs=2)[:, :260]
                for xc in range(8):
                    nc.tensor.matmul(
                        ps,
                        lhsT=xt_sb[xc][:, st * 128 : (st + 1) * 128],
                        rhs=wv_sb[xc],
                        start=(xc == 0),
                        stop=(xc == 7),
                    )
                    yield
                nc.vector.tensor_copy(out=v_sb[st], in_=ps)
                ones_cols = v_sb[st].rearrange("p (h c) -> p h c", c=65)
                nc.vector.memset(ones_cols[:, :, 64:65], 1.0)

            def out_proj(qb, evict_act=False):
                o_sb = work2.tile([128, HID], bf16, tag="osb", name="osb", bufs=3)
                for nn in range(2):
                    ps = psum.tile([128, 512], f32, tag="fill", name="outps", bufs=2)
                    for hd in range(2):
                        nc.tensor.matmul(
                            ps,
                            lhsT=mixT_sb[hd][:, qb * 128 : (qb + 1) * 128],
                            rhs=wout_sb[hd][:, nn * 512 : (nn + 1) * 512],
                            start=(hd == 0),
                            stop=(hd == 1),
                        )
                        yield
                    dst = o_sb[:, nn * 512 : (nn + 1) * 512]
                    if evict_act:
                        nc.scalar.activation(out=dst, in_=ps, func=Copy)
                    else:
                        nc.vector.tensor_copy(out=dst, in_=ps)
                nc.sync.dma_start(out=out[qb * 128 : (qb + 1) * 128, :], in_=o_sb)

            fillers = []

            def enqueue(gen):
                fillers.append(gen)

            def fill_step(n):
                for _ in range(n):
                    while fillers:
                        try:
                            next(fillers[0])
                            break
                        except StopIteration:
                            fillers.pop(0)
                    else:
                        return

            def run_block(gen):
                for _ in gen:
                    pass

            def flush():
                while fillers:
                    run_block(fillers.pop(0))

            def attention(h, qs, pops=1):
                hp, pb = h // 2, 64 * (h % 2)
                qt, kt = qk_sb[hp], qk_sb[2 + hp]
                npair = qs // 256 + 2
                kbmax = qs // 128 + 3
                mix_ps = psum.tile([65, 512], f32, tag="mix", name="mix", bufs=2)
                for p in range(npair):
                    kb_e, kb_o = 2 * p, 2 * p + 1
                    k0_e, k0_o = 128 * kb_e, 128 * kb_o
                    pe_ = max(0, k0_e - qs)
                    po_ = max(0, k0_o - qs)
                    # two-region score band: cols [0,512)=kb_o, [512,1024)=kb_e
                    sb = psum.tile([128, 1024], f32, tag="sband", name="sband", bufs=2)
                    nc.tensor.matmul(
                        sb[:, po_:512],
                        lhsT=kt[pb : pb + 64, k0_o : k0_o + 128],
                        rhs=qt[pb : pb + 64, qs + po_ : qs + 512],
                        start=True,
                        stop=True,
                    )
                    nc.tensor.matmul(
                        sb[:, 512 + pe_ : 1024],
                        lhsT=kt[pb : pb + 64, k0_e : k0_e + 128],
                        rhs=qt[pb : pb + 64, qs + pe_ : qs + 512],
                        start=True,
                        stop=True,
                    )
                    pexp = work.tile([128, 1024], bf16, tag="pexp", name="pexp")
                    if pe_ == 0:
                        nc.scalar.activation(out=pexp[:, po_:1024], in_=sb[:, po_:1024], func=Exp)
                    else:
                        nc.scalar.activation(out=pexp[:, po_:512], in_=sb[:, po_:512], func=Exp)
                        nc.scalar.activation(
                            out=pexp[:, 512 + pe_ : 1024], in_=sb[:, 512 + pe_ : 1024], func=Exp
                        )
                    fill_step(pops)
                    # one G-multiply for both regions: overlapping strided view
                    # slot a=0 -> g[o0+j] (kb_o), a=1 -> g[o0+128+j] (kb_e)
                    gt = g_sb[h]
                    o0 = 512 + qs - k0_o
                    gv = AP(
                        tensor=gt.tensor,
                        offset=gt.offset + o0,
                        ap=[list(gt.ap[0]), [128, 2], [1, 512]],
                    )
                    pband = work.tile([128, 1024], bf16, tag="pband", name="pband")
                    pb3 = pband.rearrange("p (a b) -> p a b", b=512)
                    px3 = pexp.rearrange("p (a b) -> p a b", b=512)
                    nc.vector.tensor_mul(pb3, px3, gv)
                    fill_step(1)
                    # PV: middle diagonal blocks skip their zero q-prefix; the
                    # group's first (kb 0) and last (kbmax) stay full-width so
                    # PSUM start/stop cover every element.
                    pvs_e = 0 if kb_e in (0, kbmax) else pe_
                    pvs_o = 0 if kb_o in (0, kbmax) else po_
                    nc.tensor.matmul(
                        mix_ps[:, pvs_e:],
                        lhsT=v_sb[kb_e][:, 65 * h : 65 * h + 65],
                        rhs=pband[:, 512 + pvs_e : 1024],
                        start=(kb_e == 0),
                        stop=False,
                    )
                    nc.tensor.matmul(
                        mix_ps[:, pvs_o:],
                        lhsT=v_sb[kb_o][:, 65 * h : 65 * h + 65],
                        rhs=pband[:, pvs_o:512],
                        start=False,
                        stop=(kb_o == kbmax),
                    )
                # softmax normalization: recip of rowsums (mix_ps row 64).
                row_sb = work2.tile([1, 512], f32, tag="row", name="row")
                nc.vector.tensor_copy(out=row_sb, in_=mix_ps[64:65, :])
                if h == 3 and qs == 1536:
                    # Final chain gates the kernel tail: route it through the
                    # (idle by now) PE instead of ~10us of DRAM DMA latency.
                    # wrap [1,512]->[128,4] via 4 PE transposes, recip on 128
                    # lanes, transpose back, then ones-matmul partition-bcast.
                    tp_ps = psum.tile([128, 1024], f32, tag="sband", name="tp", bufs=2)
                    for j in range(4):
                        nc.tensor.transpose(
                            tp_ps[:, j : j + 1],
                            row_sb[:, 128 * j : 128 * (j + 1)],
                            ident128[0:1, 0:1],
                        )
                    rc_sb = work2.tile([128, 4], f32, tag="rc", name="rc")
                    nc.vector.reciprocal(out=rc_sb, in_=tp_ps[:, 0:4])
                    # transpose each column separately so every PSUM read
                    # stays at partition 0 (partition-sliced PSUM reads fail
                    # the bir verifier when they lower past a bank boundary)
                    ut_ps = psum.tile([128, 1024], f32, tag="sband", name="ut", bufs=2)
                    for j in range(4):
                        nc.tensor.transpose(
                            ut_ps[0:1, 128 * j : 128 * (j + 1)],
                            rc_sb[:, j : j + 1],
                            ident128,
                        )
                    rrow_sb = work2.tile([1, 512], f32, tag="rrow", name="rrow")
                    nc.vector.tensor_copy(out=rrow_sb, in_=ut_ps[0:1, 0:512])
                    r_ps = psum.tile([128, 1024], f32, tag="sband", name="rps", bufs=2)
                    nc.tensor.matmul(
                        r_ps[0:64, 0:512], lhsT=ones64, rhs=rrow_sb, start=True, stop=True
                    )
                    r_sb = work2.tile([64, 512], f32, tag="rbc", name="rbc")
                    nc.vector.tensor_copy(out=r_sb, in_=r_ps[0:64, 0:512])
                else:
                    # DRAM bounce wraps [1,512] to [128,4] so the reciprocal
                    # runs on 128 lanes (a [1,512] reciprocal costs 3.3us on
                    # one).  Chains alternate SP/GpSimd DMA queues so
                    # consecutive heads' chains overlap.
                    dma_eng = nc.sync if state["chain"] % 2 == 0 else nc.gpsimd
                    state["chain"] += 1
                    d_s = dpool.tile([1, 512], f32, tag="ds", name="ds")
                    dma_eng.dma_start(out=d_s, in_=row_sb)
                    rs_sb = work2.tile([128, 4], f32, tag="rs", name="rs")
                    dma_eng.dma_start(out=rs_sb, in_=d_s.rearrange("o (a b) -> (o a) b", a=128))
                    rc_sb = work2.tile([128, 4], f32, tag="rc", name="rc")
                    nc.vector.reciprocal(out=rc_sb, in_=rs_sb)
                    d_r = dpool.tile([1, 512], f32, tag="dr", name="dr")
                    dma_eng.dma_start(out=d_r.rearrange("o (a b) -> (o a) b", a=128), in_=rc_sb)
                    r_sb = work2.tile([64, 512], f32, tag="rbc", name="rbc")
                    dma_eng.dma_start(out=r_sb, in_=d_r.to_broadcast([64, 512]))
                # 64-partition DVE ops may write either partition half
                # (bank->quadrant routing), so odd heads write rows 64-127
                nc.vector.tensor_mul(
                    mixT_sb[hp][pb : pb + 64, qs : qs + 512], mix_ps[0:64, :], r_sb
                )

            # ---------------- schedule (qs-major, pair-level fillers) --------
            # flush() before every segment that consumes queued filler output
            run_block(qk_pass(0, 0, tag="sband", evict_act=True))
            run_block(qk_pass(2, 0, tag="fill", evict_act=True))
            for st in range(4):
                run_block(v_pass(st, tag="sband" if st % 2 == 0 else "fill"))
            enqueue(v_pass(4))
            enqueue(v_pass(5))
            attention(0, 0)
            attention(1, 0)
            flush()
            run_block(qk_pass(1, 0, tag="fill"))
            run_block(qk_pass(3, 0, tag="fill"))
            enqueue(v_pass(6))
            enqueue(v_pass(7))
            attention(2, 0)
            attention(3, 0)
            flush()  # V0-7 complete for qs=512
            for st in range(8, 12):
                enqueue(v_pass(st))
            enqueue(qk_pass(0, 1))
            enqueue(qk_pass(2, 1))
            attention(0, 512)
            attention(1, 512)
            attention(2, 512)
            attention(3, 512)
            flush()  # V8-11 + qk halves complete for qs=1024 (h0/h1)
            enqueue(qk_pass(1, 1))
            enqueue(qk_pass(3, 1))
            attention(0, 1024)
            attention(1, 1024)
            flush()  # h2/h3 qk halves complete
            for st in range(12, 16):
                enqueue(v_pass(st))
            attention(2, 1024)
            attention(3, 1024)
            flush()  # V12-15 complete for qs=1536
            for qb in range(0, 12):
                enqueue(out_proj(qb))
            attention(0, 1536, pops=2)
            attention(1, 1536, pops=2)
            attention(2, 1536, pops=2)
            attention(3, 1536, pops=2)
            flush()
            for qb in range(12, 16):
                run_block(out_proj(qb, evict_act=(qb % 2 == 0)))
    nc.finalize()
    return nc


def host_prep(x, qkv, out_w, rpe_bias):
    """Build per-core input shards (all host work is O(N*S) or a transpose)."""
    x = np.asarray(x, np.float32)
    qkv = np.asarray(qkv, np.float32)
    out_w = np.asarray(out_w, np.float32)
    rpe_bias = np.asarray(rpe_bias, np.float32)

    xT = [np.ascontiguousarray(x[b].T).astype(BF16) for b in range(B)]  # [HID,S]

    shards = []
    for hg in range(4):
        hs = slice(hg * 4, hg * 4 + 4)
        wq = qkv[:, 0, hs, :].reshape(HID, 256) * (D ** -0.5)
        wk = qkv[:, 1, hs, :].reshape(HID, 256)
        w_qk = np.concatenate([wq, wk], axis=1).astype(BF16)

        w_v = np.zeros((HID, 260), np.float32)
        for i in range(4):
            w_v[:, i * 65 : i * 65 + 64] = qkv[:, 2, hg * 4 + i, :]
        w_v = w_v.astype(BF16)

        g = np.zeros((HPC, 128, 512 + S), np.float32)
        idx = np.arange(512 + S)[None, :] - 512 - np.arange(128)[:, None]
        valid = (idx >= 0) & (idx < S)
        for i in range(4):
            cum = np.cumsum(rpe_bias[hg * 4 + i])
            gh = np.exp(cum)
            g[i] = np.where(valid, gh[np.clip(idx, 0, S - 1)], 0.0)
        g = g.astype(BF16)

        w_o = out_w[hs].reshape(256, HID).astype(BF16)
        shards.append((w_qk, w_v, g, w_o))

    in_maps = []
    for c in range(NCORES):
        b, hg = c // 4, c % 4
        w_qk, w_v, g, w_o = shards[hg]
        in_maps.append(
            {"x_t": xT[b], "w_qk": w_qk, "w_v": w_v, "g_ext": g, "w_out": w_o}
        )
    return in_maps


def run(in_maps, trace=False):
    from concourse.bass_utils import run_bass_kernel_spmd

    if "nc" not in _CACHE:
        _CACHE["nc"] = build_nc()
    nc = _CACHE["nc"]
    res = run_bass_kernel_spmd(nc, in_maps, core_ids=list(range(NCORES)), trace=trace)
    return res


def kernel(x, qkv, out_w, rpe_bias):
    in_maps = host_prep(x, qkv, out_w, rpe_bias)
    res = run(in_maps)
    parts = [np.asarray(res.results[c]["out"], np.float32) for c in range(NCORES)]
    out = np.stack(
        [
            parts[0] + parts[1] + parts[2] + parts[3],
            parts[4] + parts[5] + parts[6] + parts[7],
        ]
    ).astype(np.float32)
    return out


if __name__ == "__main__":
    nc = build_nc()
    print("built ok")
